# revision 11
# baseline (speedup 1.0000x reference)
"""Trainium2 Bass kernel for nn_EnsembleModel (embedding_lookup ensemble loss).

Strategy (8 cores, data/table hybrid sharding):
  - simi_score_mtx row-sharded 1818 rows/core; each core streams its shard in
    [<=128, 4847] tiles and row-sums on DVE; AllGather of per-core row sums.
  - stelp_ent_emb row-sharded the same way; per-sample sum / sum-of-squares of
    gathered rows computed as count-matrix matmuls on PE (host builds the
    integer count matrix from ent_idx); ReduceScatter hands each core the
    totals for its own 16 samples -> unbiased std.
  - The simi gather + dot with proj_w's simi segment is folded into a
    host-built scatter matrix W1 (entries proj_w[768+j]/N_ENT at the slot of
    ent_idx[b,j]) -> small matmuls against the all-gathered row sums.
  - Remaining feature dots, sigmoid, and the margin loss run on [16, *] tiles;
    each core emits a partial loss sum, host combines.
"""

import os
import sys

for _p in ("/opt/trn_rl_repo", "/root/.axon_site/_ro/trn_rl_repo"):
    if os.path.isdir(_p) and _p not in sys.path:
        sys.path.insert(0, _p)

import numpy as np

import concourse.bacc as bacc
import concourse.bass as bass
import concourse.mybir as mybir
import concourse.tile as tile
from concourse.bass_utils import run_bass_kernel_spmd

F32 = mybir.dt.float32
X = mybir.AxisListType.X
AF = mybir.ActivationFunctionType

N_ENT = 14541
EMB = 768
TOPK = 1000
NEG = 5
BS = 128
NCORES = 8
BSL = BS // NCORES          # 16 samples per core
MARGIN = 0.5

RS = 1818                   # simi/emb rows per core (8*1818 = 14544 >= 14541)
RSP = 1920                  # padded row-sum slots per core (15*128)
RM_TOT = RSP * NCORES       # 15360 = 128*120
WCH = RM_TOT // 128         # 120 W1 chunks
ECH = 15                    # emb chunks per core: 14*128 + 26
E_TAIL = RS - 14 * 128      # 26
FH = 4847                   # 14541 = 3*4847
ST = 15                     # simi row tiles per core (14*128 + 26)
S_TAIL = RS - 14 * 128      # 26

_CACHE = {}


def _build():
    nc = bacc.Bacc("TRN2", target_bir_lowering=False, debug=False,
                   num_devices=NCORES)

    simi = nc.dram_tensor("simi_shard", [RS, 3 * FH], F32, kind="ExternalInput")
    embt = nc.dram_tensor("emb_shard", [RS, EMB], F32, kind="ExternalInput")
    cbuf = nc.dram_tensor("c_buf", [128, ECH * 128], F32, kind="ExternalInput")
    w1buf = nc.dram_tensor("w1_buf", [128, WCH * BSL], F32, kind="ExternalInput")
    wmisc = nc.dram_tensor("w_misc", [BSL, 4 * TOPK + EMB], F32, kind="ExternalInput")
    st_in = nc.dram_tensor("st_loc", [BSL, TOPK], F32, kind="ExternalInput")
    rot_in = nc.dram_tensor("rot_loc", [BSL, TOPK], F32, kind="ExternalInput")
    pos_st_in = nc.dram_tensor("pos_st", [BSL, 1], F32, kind="ExternalInput")
    pos_rot_in = nc.dram_tensor("pos_rot", [BSL, 1], F32, kind="ExternalInput")
    neg_st_in = nc.dram_tensor("neg_st", [BSL, NEG], F32, kind="ExternalInput")
    neg_rot_in = nc.dram_tensor("neg_rot", [BSL, NEG], F32, kind="ExternalInput")
    projb_in = nc.dram_tensor("projb", [BSL, 1], F32, kind="ExternalInput")
    ones_in = nc.dram_tensor("ones16", [BSL, 1], F32, kind="ExternalInput")
    margin_in = nc.dram_tensor("margin16", [BSL, 1], F32, kind="ExternalInput")

    out_loss = nc.dram_tensor("loss_partial", [1, 1], F32, kind="ExternalOutput")

    groups = [list(range(NCORES))]

    with tile.TileContext(nc) as tc:
        with (
            tc.tile_pool(name="p_simi", bufs=3) as p_simi,
            tc.tile_pool(name="p_emb", bufs=6) as p_emb,
            tc.tile_pool(name="p_const", bufs=1) as p_const,
            tc.tile_pool(name="p_ps", bufs=1, space="PSUM") as p_ps,
            tc.tile_pool(name="p_dram", bufs=1, space="DRAM") as p_dram,
        ):
            # ---- constant / small loads (ACT HWDGE ring) ----
            c_sb = p_const.tile([128, ECH * 128], F32)
            nc.scalar.dma_start(c_sb[:], cbuf.ap())
            w1_sb = p_const.tile([128, WCH * BSL], F32)
            nc.scalar.dma_start(w1_sb[:], w1buf.ap())
            wm_sb = p_const.tile([BSL, 4 * TOPK + EMB], F32)
            nc.scalar.dma_start(wm_sb[:], wmisc.ap())
            feat = p_const.tile([BSL, 4 * TOPK + EMB], F32)
            nc.scalar.dma_start(feat[:, 2 * TOPK + EMB:3 * TOPK + EMB], st_in.ap())
            nc.scalar.dma_start(feat[:, 3 * TOPK + EMB:4 * TOPK + EMB], rot_in.ap())
            pos_st = p_const.tile([BSL, 1], F32)
            nc.scalar.dma_start(pos_st[:], pos_st_in.ap())
            pos_rot = p_const.tile([BSL, 1], F32)
            nc.scalar.dma_start(pos_rot[:], pos_rot_in.ap())
            neg_st = p_const.tile([BSL, NEG], F32)
            nc.scalar.dma_start(neg_st[:], neg_st_in.ap())
            neg_rot = p_const.tile([BSL, NEG], F32)
            nc.scalar.dma_start(neg_rot[:], neg_rot_in.ap())
            projb = p_const.tile([BSL, 1], F32)
            nc.scalar.dma_start(projb[:], projb_in.ap())
            ones16 = p_const.tile([BSL, 1], F32)
            nc.scalar.dma_start(ones16[:], ones_in.ap())
            margin16 = p_const.tile([BSL, 1], F32)
            nc.scalar.dma_start(margin16[:], margin_in.ap())

            # ---- emb phase: per-sample sum / sumsq over the table shard ----
            ps_s1 = p_ps.tile([128, 384], F32, space="PSUM")
            ps_s2 = p_ps.tile([128, 384], F32, space="PSUM")
            ps_q1 = p_ps.tile([128, 384], F32, space="PSUM")
            ps_q2 = p_ps.tile([128, 384], F32, space="PSUM")
            for ci in range(ECH):
                rows = 128 if ci < ECH - 1 else E_TAIL
                et = p_emb.tile([128, EMB], F32)
                nc.scalar.dma_start(et[:rows, :],
                                    embt.ap()[ci * 128:ci * 128 + rows, :])
                lhs = c_sb[:rows, ci * 128:(ci + 1) * 128]
                st_flag = (ci == 0)
                sp_flag = (ci == ECH - 1)
                nc.tensor.matmul(out=ps_s1[:], lhsT=lhs, rhs=et[:rows, 0:384],
                                 start=st_flag, stop=sp_flag)
                nc.tensor.matmul(out=ps_s2[:], lhsT=lhs, rhs=et[:rows, 384:768],
                                 start=st_flag, stop=sp_flag)
                nc.scalar.square(et[:rows, :], et[:rows, :])
                nc.tensor.matmul(out=ps_q1[:], lhsT=lhs, rhs=et[:rows, 0:384],
                                 start=st_flag, stop=sp_flag)
                nc.tensor.matmul(out=ps_q2[:], lhsT=lhs, rhs=et[:rows, 384:768],
                                 start=st_flag, stop=sp_flag)

            bs_sum = p_const.tile([128, EMB], F32)
            nc.scalar.copy(bs_sum[:, 0:384], ps_s1[:])
            nc.scalar.copy(bs_sum[:, 384:768], ps_s2[:])
            bs_sq = p_const.tile([128, EMB], F32)
            nc.scalar.copy(bs_sq[:, 0:384], ps_q1[:])
            nc.scalar.copy(bs_sq[:, 384:768], ps_q2[:])

            rs_sum_in = p_dram.tile([128, EMB], F32)
            rs_sq_in = p_dram.tile([128, EMB], F32)
            rs_sum_out = p_dram.tile([BSL, EMB], F32)
            rs_sq_out = p_dram.tile([BSL, EMB], F32)
            nc.scalar.dma_start(rs_sum_in[:], bs_sum[:])
            nc.scalar.dma_start(rs_sq_in[:], bs_sq[:])
            nc.gpsimd.collective_compute(
                "ReduceScatter", mybir.AluOpType.add, replica_groups=groups,
                ins=[rs_sum_in.opt()], outs=[rs_sum_out.opt()])
            nc.gpsimd.collective_compute(
                "ReduceScatter", mybir.AluOpType.add, replica_groups=groups,
                ins=[rs_sq_in.opt()], outs=[rs_sq_out.opt()])

            # ---- simi phase: row sums of the 1818x14541 shard ----
            # two ~3.7 MB DMAs per 128-row tile (14541 = 7271 + 7270)
            rm3a = p_const.tile([128, 2 * ST], F32)
            nc.vector.memset(rm3a[:], 0.0)
            for t in range(ST):
                rows = 128 if t < ST - 1 else S_TAIL
                for h, (off, w) in enumerate([(0, 7271), (7271, 7270)]):
                    stile = p_simi.tile([128, 7296], F32)
                    nc.sync.dma_start(
                        stile[:rows, :w],
                        simi.ap()[t * 128:t * 128 + rows, off:off + w])
                    nc.vector.reduce_sum(rm3a[:rows, 2 * t + h:2 * t + h + 1],
                                         stile[:rows, :w], axis=X)

            rm_sb = p_const.tile([128, ST], F32)
            nc.vector.tensor_add(rm_sb[:], rm3a[:, 0:2 * ST:2], rm3a[:, 1:2 * ST:2])

            rm_local = p_dram.tile([1, RSP], F32)
            nc.sync.dma_start(
                rm_local[:].rearrange("a (t p) -> p (a t)", p=128), rm_sb[:])
            rm_full = p_dram.tile([1, RM_TOT], F32)
            nc.gpsimd.collective_compute(
                "AllGather", mybir.AluOpType.bypass, replica_groups=groups,
                ins=[rm_local.opt()], outs=[rm_full.opt()])

            # ---- W1 @ row-sums -> simi logit contribution ----
            rm_ld = p_const.tile([128, WCH], F32)
            nc.sync.dma_start(
                rm_ld[:], rm_full[:].rearrange("a (p c) -> p (a c)", c=WCH))
            ps_l = p_ps.tile([BSL, 1], F32, space="PSUM")
            for c in range(WCH):
                nc.tensor.matmul(out=ps_l[:],
                                 lhsT=w1_sb[:, c * BSL:(c + 1) * BSL],
                                 rhs=rm_ld[:, c:c + 1],
                                 start=(c == 0), stop=(c == WCH - 1))

            # ---- emb std -> feat[:, :EMB] ----
            sum_sb = p_const.tile([BSL, EMB], F32)
            nc.scalar.dma_start(sum_sb[:], rs_sum_out[:])
            sq_sb = p_const.tile([BSL, EMB], F32)
            nc.scalar.dma_start(sq_sb[:], rs_sq_out[:])
            t1 = p_const.tile([BSL, EMB], F32)
            nc.vector.tensor_mul(t1[:], sum_sb[:], sum_sb[:])
            nc.vector.tensor_scalar_mul(t1[:], t1[:], 1.0 / TOPK)
            nc.vector.tensor_sub(t1[:], sq_sb[:], t1[:])
            nc.vector.tensor_scalar_max(t1[:], t1[:], 0.0)
            nc.scalar.activation(feat[:, 0:EMB], t1[:], AF.Sqrt,
                                 scale=1.0 / (TOPK - 1))

            # ---- score features ----
            o_st = 2 * TOPK + EMB
            o_rot = 3 * TOPK + EMB
            nc.vector.tensor_sub(feat[:, EMB:EMB + TOPK],
                                 feat[:, o_rot:o_rot + TOPK],
                                 feat[:, o_st:o_st + TOPK])
            nc.scalar.activation(feat[:, EMB:EMB + TOPK],
                                 feat[:, EMB:EMB + TOPK], AF.Abs)
            nc.vector.tensor_add(feat[:, EMB + TOPK:EMB + 2 * TOPK],
                                 feat[:, o_rot:o_rot + TOPK],
                                 feat[:, o_st:o_st + TOPK])

            lmisc = p_const.tile([BSL, 1], F32)
            nc.vector.tensor_mul(feat[:], feat[:], wm_sb[:])
            nc.vector.reduce_sum(lmisc[:], feat[:], axis=X)

            # ---- alpha, ensemble scores, loss ----
            logit = p_const.tile([BSL, 1], F32)
            nc.vector.tensor_add(logit[:], lmisc[:], ps_l[:])
            alpha = p_const.tile([BSL, 1], F32)
            nc.scalar.activation(alpha[:], logit[:], AF.Sigmoid, bias=projb[:, :])

            d1 = p_const.tile([BSL, 1], F32)
            nc.vector.tensor_sub(d1[:], pos_st[:], pos_rot[:])
            nc.vector.tensor_mul(d1[:], d1[:], alpha[:])
            nc.vector.tensor_add(d1[:], d1[:], pos_rot[:])   # pos_ens

            d5 = p_const.tile([BSL, NEG], F32)
            nc.vector.tensor_sub(d5[:], neg_st[:], neg_rot[:])
            nc.vector.tensor_scalar_mul(d5[:], d5[:], alpha[:, :])
            nc.vector.tensor_add(d5[:], d5[:], neg_rot[:])   # neg_ens
            nc.vector.tensor_scalar(out=d5[:], in0=d5[:], scalar1=d1[:, :],
                                    scalar2=None, op0=mybir.AluOpType.subtract)
            row_loss = p_const.tile([BSL, 1], F32)
            nc.scalar.activation(d5[:], d5[:], AF.Relu, bias=margin16[:, :],
                                 accum_out=row_loss[:])

            ps_f = p_ps.tile([1, 1], F32, space="PSUM")
            nc.tensor.matmul(out=ps_f[:], lhsT=ones16[:], rhs=row_loss[:],
                             start=True, stop=True)
            fin = p_const.tile([1, 1], F32)
            nc.vector.tensor_copy(fin[:], ps_f[:])
            nc.sync.dma_start(out_loss.ap(), fin[:])

    nc.compile()
    return nc


def _prep_inputs(inputs):
    idx = np.asarray(inputs["ent_idx"])
    simi = np.ascontiguousarray(np.asarray(inputs["simi_score_mtx"], dtype=np.float32))
    emb = np.ascontiguousarray(np.asarray(inputs["stelp_ent_emb"], dtype=np.float32))
    projw = np.asarray(inputs["proj_w"], dtype=np.float32).reshape(-1)
    projb = float(np.asarray(inputs["proj_b"], dtype=np.float32).reshape(-1)[0])
    st = np.asarray(inputs["stelp_scores"], dtype=np.float32)
    rot = np.asarray(inputs["rotate_scores"], dtype=np.float32)
    pos_st = np.asarray(inputs["pos_stelp_score"], dtype=np.float32).reshape(BS, 1)
    pos_rot = np.asarray(inputs["pos_rotate_score"], dtype=np.float32).reshape(BS, 1)
    neg_st = np.asarray(inputs["neg_stelp_scores"], dtype=np.float32)
    neg_rot = np.asarray(inputs["neg_rotate_scores"], dtype=np.float32)

    w_emb = projw[0:EMB]
    w_simi = projw[EMB:EMB + TOPK]
    w_misc_row = np.concatenate([w_emb, projw[EMB + TOPK:]]).astype(np.float32)
    w_misc = np.ascontiguousarray(np.broadcast_to(w_misc_row, (BSL, w_misc_row.size)))

    ones16 = np.ones((BSL, 1), np.float32)
    projb16 = np.full((BSL, 1), projb, np.float32)

    b_glob = np.broadcast_to(np.arange(BS)[:, None], (BS, TOPK)).ravel()
    e_flat = idx.ravel().astype(np.int64)

    in_maps = []
    for c in range(NCORES):
        r0 = c * RS
        r1 = min(r0 + RS, N_ENT)
        if r1 - r0 == RS:
            simi_c = simi[r0:r1]
            emb_c = emb[r0:r1]
        else:
            pad = RS - (r1 - r0)
            simi_c = np.vstack([simi[r0:r1], np.zeros((pad, N_ENT), np.float32)])
            emb_c = np.vstack([emb[r0:r1], np.zeros((pad, EMB), np.float32)])

        # count matrix over this core's entity rows, all 128 samples
        m = (e_flat >= r0) & (e_flat < r0 + RS)
        el = e_flat[m] - r0
        cb = np.zeros((128, ECH * 128), np.float32)
        np.add.at(cb, (el % 128, (el // 128) * 128 + b_glob[m]), 1.0)

        # W1 scatter for this core's 16 samples (simi segment of proj_w / N_ENT)
        idx_c = idx[c * BSL:(c + 1) * BSL].astype(np.int64)
        s = (idx_c // RS) * RSP + (idx_c % RS)          # [16, 1000] slots
        vals = np.broadcast_to(w_simi / float(N_ENT), (BSL, TOPK))
        b_loc = np.broadcast_to(np.arange(BSL)[:, None], (BSL, TOPK))
        w1 = np.zeros((128, WCH * BSL), np.float64)
        np.add.at(w1, ((s // WCH).ravel(), ((s % WCH) * BSL + b_loc).ravel()),
                  vals.ravel())
        w1 = w1.astype(np.float32)

        in_maps.append({
            "simi_shard": simi_c,
            "emb_shard": emb_c,
            "c_buf": cb,
            "w1_buf": w1,
            "w_misc": w_misc,
            "st_loc": np.ascontiguousarray(st[c * BSL:(c + 1) * BSL]),
            "rot_loc": np.ascontiguousarray(rot[c * BSL:(c + 1) * BSL]),
            "pos_st": np.ascontiguousarray(pos_st[c * BSL:(c + 1) * BSL]),
            "pos_rot": np.ascontiguousarray(pos_rot[c * BSL:(c + 1) * BSL]),
            "neg_st": np.ascontiguousarray(neg_st[c * BSL:(c + 1) * BSL]),
            "neg_rot": np.ascontiguousarray(neg_rot[c * BSL:(c + 1) * BSL]),
            "projb": projb16,
            "ones16": ones16,
            "margin16": np.full((BSL, 1), MARGIN, np.float32),
        })
    return in_maps


def kernel(**inputs) -> np.ndarray:
    if "nc" not in _CACHE:
        _CACHE["nc"] = _build()
    nc = _CACHE["nc"]
    in_maps = _prep_inputs(inputs)
    res = run_bass_kernel_spmd(nc, in_maps, core_ids=list(range(NCORES)))
    total = sum(float(res.results[c]["loss_partial"][0, 0]) for c in range(NCORES))
    return np.array(np.float32(total / (BS * NEG)))
